# revision 53
# baseline (speedup 1.0000x reference)
"""Trainium2 Bass kernel for nn_BasicBlock (FBS-masked ternary conv + BN + LeakyReLU).

Sharding: data-parallel over batch. B=32 -> 4 samples per core on 8 cores.
BN batch stats are AllGathered (cheaper than AllReduce) and reduced locally.

v2 design (vs baseline):
  - W loaded ONCE in its fast natural layout [co, (ci kh kw)] (16KB contiguous
    runs -> full DMA bw), ternarized in that layout to bf16, then transposed
    on the PE (idle during the prologue) into the [ci, (khw t), co] lhsT
    layout.  Threshold compares stay f32 (bf16 compare flips ~0.15% of
    weights near +-t -> ~4% output error, NOT ok).
  - x loaded ONCE; quadrant-interleaved (bf16) for all 4 samples resident.
  - Conv matmuls in bf16 (cost model: 1 cyc/row same as f32r, half SBUF).
  - y kept in SBUF (bf16) - no DRAM round trip.
  - Conv loop cot-major; per-cot BN stats AllGather + epilogue overlap the
    conv of later cots.  Epilogue (y*scl+shf, leaky relu) runs on the DVE
    (max(z, 0.2*z)) so the Activation engine only handles evictions.
  - Exact top-k threshold per sample, pipelined per sample so masks are
    ready before the first eviction; broadcasts via tiny PE matmuls instead
    of DRAM bounces.
"""

import numpy as np

import concourse.bass as bass
import concourse.mybir as mybir
import concourse.tile as tile
from concourse.bass_utils import run_bass_kernel_spmd
from concourse.masks import make_identity

F32 = mybir.dt.float32
F16 = mybir.dt.float16
F8 = mybir.dt.float8e4
BF16 = mybir.dt.bfloat16
AF = mybir.ActivationFunctionType
ALU = mybir.AluOpType
AX = mybir.AxisListType

N_CORES = 8
B, CIN, H, W = 32, 256, 64, 64
COUT, KK = 512, 4
OH, OW = 32, 32
NB = B // N_CORES          # samples per core = 4
NT = CIN // 128            # ci tiles = 2
NCOT = COUT // 128         # co tiles = 4
CR_KEEP = 409.5            # count <= 409  <->  count < 409.5
BN_EPS = 1e-5
NEG_SLOPE = 0.2
THRESH_FACTOR = 0.05
NSP = OH * OW              # 1024 spatial positions per sample
BIG = 1.0e30

MAX_WAITS = 1              # this walrus build allows 1 sync wait per instruction

# kh -> (row parity ph, row shift dj): x row 2*oh + kh - 1 = 2*(oh+dj) + ph
PAR = {0: (1, -1), 1: (0, 0), 2: (1, 0), 3: (0, 1)}
KHW_ORDER = ([(1, 1)]
             + [(kh, kw) for kh in range(KK) for kw in range(KK)
                if (kh, kw) != (1, 1) and kh * KK + kw < 8]
             + [(kh, kw) for kh in range(KK) for kw in range(KK)
                if kh * KK + kw >= 8])


def _split_waits(nc, max_waits=MAX_WAITS):
    """Split per-instruction sem waits exceeding max_waits into preceding
    same-engine InstNoOp carriers (engines execute their queue in order)."""
    for f in nc.m.functions:
        for bb in f.blocks:
            new_list = []
            changed = False
            for ins in bb.instructions:
                si = ins.sync_info
                if si is not None and si.on_wait and len(si.on_wait) > max_waits:
                    waits = list(si.on_wait)
                    carry = waits[: len(waits) - max_waits]
                    keep = waits[len(waits) - max_waits:]
                    k = 0
                    while carry:
                        chunk, carry = carry[:max_waits], carry[max_waits:]
                        new_list.append(
                            mybir.InstNoOp(
                                name=f"{ins.name}_ws{k}",
                                engine=ins.engine,
                                bass_nofuse=True,
                                sync_info=mybir.SyncInfo(on_wait=chunk, on_update=[]),
                            )
                        )
                        k += 1
                    ins.sync_info = mybir.SyncInfo(
                        on_wait=keep, on_update=list(si.on_update)
                    )
                    changed = True
                new_list.append(ins)
            if changed:
                bb.instructions = new_list


def build_kernel(r_imm: float, eps_imm: float, debug: bool = False):
    """Build the per-core Bass module. r_imm = neg/pos, eps_imm = eps/pos^2."""
    nc = bass.Bass()

    xs = nc.dram_tensor("xs", [NB, CIN, H, W], F32, kind="ExternalInput")
    wt = nc.dram_tensor("wt", [COUT, CIN, KK, KK], F32, kind="ExternalInput")
    salw = nc.dram_tensor("salw", [COUT, CIN], F32, kind="ExternalInput")
    salb = nc.dram_tensor("salb", [COUT], F32, kind="ExternalInput")
    gam = nc.dram_tensor("gam", [COUT], F32, kind="ExternalInput")
    bet = nc.dram_tensor("bet", [COUT], F32, kind="ExternalInput")
    out = nc.dram_tensor("out", [NB, COUT, OH, OW], F32, kind="ExternalOutput")

    cc_out = nc.dram_tensor("cc_out", [NCOT, N_CORES, 2, 128], F32,
                            addr_space="Shared")
    if debug:
        dbg_sub = nc.dram_tensor("dbg_sub", [128, NT * NB], F32, kind="ExternalOutput")
        dbg_sal = nc.dram_tensor("dbg_sal", [128, NCOT * NB], F32, kind="ExternalOutput")
        dbg_mask = nc.dram_tensor("dbg_mask", [128, NCOT * NB], F32, kind="ExternalOutput")
        dbg_bc = nc.dram_tensor("dbg_bc", [128, COUT], F32, kind="ExternalOutput")
        dbg_thr = nc.dram_tensor("dbg_thr", [NB], F32, kind="ExternalOutput")
        dbg_s1 = nc.dram_tensor("dbg_s1", [NCOT, 128, 2 * NB], F32, kind="ExternalOutput")
        dbg_s2 = nc.dram_tensor("dbg_s2", [NCOT, 128, 2 * NB], F32, kind="ExternalOutput")
        dbg_wq = nc.dram_tensor("dbg_wq", [128, 32 * 128], BF16, kind="ExternalOutput")
        dbg_scl = nc.dram_tensor("dbg_scl", [128, NCOT], F32, kind="ExternalOutput")
        dbg_shf = nc.dram_tensor("dbg_shf", [128, NCOT], F32, kind="ExternalOutput")

    with tile.TileContext(nc) as tc:
        with (
            tc.tile_pool(name="persist", bufs=1) as pp,
            tc.tile_pool(name="big", bufs=4) as bigp,
            tc.tile_pool(name="wq", bufs=2) as wqp,
            tc.tile_pool(name="stage", bufs=3) as stp,
            tc.tile_pool(name="small", bufs=2) as smp,
            tc.tile_pool(name="ps", bufs=8, space="PSUM") as psp,
            tc.tile_pool(name="dram", bufs=1, space="DRAM") as dp,
        ):
            # ---------- constants ----------
            identF = pp.tile([128, 128], F32, name="identF")
            make_identity(nc, identF)
            identB = pp.tile([128, 128], BF16, name="identB")
            make_identity(nc, identB)
            onesP1 = pp.tile([1, 128], F32, name="onesP1")
            nc.vector.memset(onesP1, 1.0)
            ones1 = pp.tile([128, 1], BF16, name="ones1")
            nc.vector.memset(ones1, 1.0)
            epst = pp.tile([128, 1], F32, name="epst")
            nc.vector.memset(epst, float(eps_imm))

            # ---------- DMA emission: W chunks, salw, then x ----------
            wch = []
            for c in range(NCOT):
                wc = bigp.tile([128, NT * 128 * 16], F32, name=f"wch{c}", tag="big")
                nc.sync.dma_start(
                    out=wc,
                    in_=wt[c * 128:(c + 1) * 128, :, :, :].rearrange(
                        "co ci kh kw -> co (ci kh kw)"))
                wch.append(wc)

            swn = []
            for c in range(NCOT):
                sw = smp.tile([128, CIN], F32, name=f"swn{c}", tag="salw")
                nc.sync.dma_start(out=sw, in_=salw[c * 128:(c + 1) * 128, :])
                swn.append(sw)

            def col128(dram_vec, nm):  # [512] dram -> [128,4] sbuf
                t_ = pp.tile([128, NCOT], F32, name=nm)
                ap = bass.AP(tensor=dram_vec, offset=0, ap=[[1, 128], [128, NCOT]])
                nc.sync.dma_start(out=t_, in_=ap)
                return t_

            salb_t = col128(salb, "salb_t")
            gam_t = col128(gam, "gam_t")
            bet_t = col128(bet, "bet_t")

            # x quadrants (bf16, all 4 samples) + |x| row sums via Act accum.
            # quads[b][t][ph][pw][ci, j, i] = x[ci, 2j+ph, 2i+pw]
            quads = [[[[pp.tile([128, OH, OW], BF16, name=f"q{b}{t}{ph}{pw}")
                        for pw in range(2)] for ph in range(2)]
                      for t in range(NT)] for b in range(NB)]
            ssc = [pp.tile([128, NT * 4], F32, name=f"ssc{b}") for b in range(NB)]
            subT = [pp.tile([128, NB], F32, name=f"subT{t}") for t in range(NT)]

            def load_sample(b, ts=None):
                for t in (range(NT) if ts is None else ts):
                    for hq in range(4):
                        stg = stp.tile([128, 16, W], F32, name=f"x{b}{t}{hq}",
                                       tag="stage")
                        nc.sync.dma_start(
                            out=stg,
                            in_=xs[b, t * 128:(t + 1) * 128,
                                   hq * 16:(hq + 1) * 16, :])
                        ascr = smp.tile([128, 16 * W], F8, name=f"as{b}{t}{hq}",
                                        tag="ascr")
                        nc.scalar.activation(
                            ascr, stg.rearrange("p a b -> p (a b)"), AF.Abs,
                            accum_out=ssc[b][:, t * 4 + hq: t * 4 + hq + 1])
                        for ph in range(2):
                            for pw in range(2):
                                nc.gpsimd.tensor_copy(
                                    out=quads[b][t][ph][pw][:,
                                                            hq * 8:(hq + 1) * 8, :],
                                    in_=stg[:, ph::2, pw::2])

            load_sample(0, ts=[0])

            # ---------- weight max: t = 0.05 * max|W| ----------
            mx = pp.tile([128, NCOT], F32, name="mx")
            for c in range(NCOT):
                nc.vector.tensor_reduce(
                    mx[:, c:c + 1], wch[c], axis=AX.X,
                    op=ALU.max, apply_absolute_value=True)
            mxr = pp.tile([128, 1], F32, name="mxr")
            nc.vector.tensor_reduce(mxr, mx, axis=AX.X, op=ALU.max)
            pb0 = psp.tile([128, 512], F32, name="pb0", tag="bank")
            nc.tensor.transpose(pb0[0:1, 0:128], mxr, identF)
            gmaxrow = pp.tile([1, 128], F32, name="gmaxrow")
            nc.scalar.copy(gmaxrow, pb0[0:1, 0:128])
            tval = pp.tile([1, 1], F32, name="tval")
            nc.vector.tensor_reduce(tval, gmaxrow, axis=AX.X, op=ALU.max)
            nc.vector.tensor_scalar(tval, tval, float(THRESH_FACTOR), None,
                                    op0=ALU.mult)
            t_d = dp.tile([1, 1], F32, name="t_d")
            nc.sync.dma_start(out=t_d, in_=tval)
            tcol = pp.tile([128, 1], F32, name="tcol")
            nc.sync.dma_start(
                out=tcol, in_=bass.AP(tensor=t_d.tensor, offset=t_d.offset,
                                      ap=[[0, 128], [1, 1]]))
            ntcol = pp.tile([128, 1], F32, name="ntcol")
            nc.vector.tensor_scalar(ntcol, tcol, -1.0, None, op0=ALU.mult)

            load_sample(0, ts=[1])
            for b in range(1, NB):
                load_sample(b)

            # ---------- salw transposes: salwT[t][ci, co] ----------
            salwT = [pp.tile([128, COUT], F32, name=f"swT{t}") for t in range(NT)]
            for c in range(NCOT):
                for t in range(NT):
                    pbt = psp.tile([128, 512], F32, name=f"ptw{c}{t}", tag="bank")
                    nc.tensor.transpose(pbt[:, 0:128],
                                        swn[c][:, t * 128:(t + 1) * 128], identF)
                    nc.scalar.copy(salwT[t][:, c * 128:(c + 1) * 128],
                                   pbt[:, 0:128])

            # ---------- ternarize + transpose (per co-chunk) ----------
            # wcht[c][co, g*128+ci'] with g = khw*2 + t holds
            #   w'[co, ci, kh, kw] = [W>t] + r*[W<-t]   (bf16, r rounded)
            # then 32 PE transposes -> wq[c][ci', g*128 + co] (lhsT layout).
            wq = []

            def tern_half(c, wcht, h):
                # khw in [8h, 8h+8): is_gt into wcht, then += r*is_lt
                src_h = bass.AP(
                    tensor=wch[c].tensor,
                    offset=wch[c].offset + h * 8,
                    ap=[wch[c].ap[0], [1, 8], [128 * 16, NT], [16, 128]])
                dsth = wcht[:, h * 8 * NT * 128:(h + 1) * 8 * NT * 128]
                dst_h = bass.AP(
                    tensor=wcht.tensor,
                    offset=wcht.offset + h * 8 * NT * 128,
                    ap=[wcht.ap[0], [NT * 128, 8], [128, NT], [1, 128]])
                nc.vector.tensor_scalar(dst_h, src_h, tcol[:, :], None,
                                        op0=ALU.is_gt)
                g2s = stp.tile([128, 8 * NT * 128], BF16, name=f"g2{c}{h}",
                               tag="g2", bufs=1)
                g2d = bass.AP(
                    tensor=g2s.tensor, offset=g2s.offset,
                    ap=[g2s.ap[0], [NT * 128, 8], [128, NT], [1, 128]])
                nc.vector.tensor_scalar(g2d, src_h, ntcol[:, :], float(r_imm),
                                        op0=ALU.is_lt, op1=ALU.mult)
                nc.vector.tensor_tensor(out=dsth, in0=dsth, in1=g2s, op=ALU.add)

            def transpose_gbs(c, wcht, wqc, gbs):
                for gb in gbs:  # 4 transposes per PSUM bank
                    pbk = psp.tile([128, 512], BF16, name=f"tw{c}{gb}", tag="bank")
                    for k in range(4):
                        g = gb * 4 + k
                        nc.tensor.transpose(
                            pbk[:, k * 128:(k + 1) * 128],
                            wcht[:, g * 128:(g + 1) * 128], identB)
                    nc.scalar.activation(
                        wqc[:, gb * 512:(gb + 1) * 512], pbk, AF.Copy)

            # chunk 0 fully in the prologue, pipelined per half
            wcht0 = smp.tile([128, NT * 128 * 16], BF16, name="wcht0",
                             tag="wcht", bufs=2)
            wq0 = wqp.tile([128, 32 * 128], BF16, name="wq0", tag="wq")
            tern_half(0, wcht0, 0)
            transpose_gbs(0, wcht0, wq0, [2, 0, 1, 3])
            tern_half(0, wcht0, 1)
            transpose_gbs(0, wcht0, wq0, range(4, 8))
            wq.append(wq0)
            # chunk 1 ternarized now; its transposes interleave with cot-0 conv
            wcht1 = smp.tile([128, NT * 128 * 16], BF16, name="wcht1",
                             tag="wcht", bufs=2)
            tern_half(1, wcht1, 0)
            tern_half(1, wcht1, 1)

            # ---------- per-sample exact top-k threshold + mask ----------
            # mask_cb[:, c*NB + b] is the per-(cot,sample) eviction scale.
            mask_cb = pp.tile([128, NCOT * NB], F32, name="mask_cb")
            thr_d = dp.tile([NB, 1], F32, name="thr_d")
            thrB = [pp.tile([128, 1], F32, name=f"thrB{b}") for b in range(NB)]
            sal_cb = [pp.tile([128, NB], F32, name=f"salcb{c}")
                      for c in range(NCOT)]
            inv_hw = 1.0 / float(H * W)

            def topk_sample(b):
                for t in range(NT):
                    nc.vector.tensor_reduce(
                        subT[t][:, b:b + 1], ssc[b][:, t * 4:(t + 1) * 4],
                        axis=AX.X, op=ALU.add)
                psal = psp.tile([128, 512], F32, name=f"psal{b}", tag="bank")
                for c in range(NCOT):
                    for t in range(NT):
                        nc.tensor.matmul(psal[:, c:c + 1],
                                         salwT[t][:, c * 128:(c + 1) * 128],
                                         subT[t][:, b:b + 1],
                                         start=(t == 0), stop=(t == NT - 1))
                    nc.scalar.activation(sal_cb[c][:, b:b + 1], psal[:, c:c + 1],
                                         AF.Abs, bias=salb_t[:, c:c + 1],
                                         scale=inv_hw)
                # salrow[0, :] = sal[b, :] ; bc = broadcast to all partitions
                prow = psp.tile([128, 512], F32, name=f"prow{b}", tag="bank")
                for c in range(NCOT):
                    nc.tensor.transpose(prow[0:1, c * 128:(c + 1) * 128],
                                        sal_cb[c][:, b:b + 1], identF)
                salrow = smp.tile([1, COUT], F32, name=f"srow{b}", tag="srow", bufs=1)
                nc.vector.tensor_copy(out=salrow, in_=prow[0:1, :])
                pbc = psp.tile([128, 512], F32, name=f"pbc{b}", tag="bank")
                nc.tensor.matmul(pbc, onesP1, salrow, start=True, stop=True)
                if debug and b == 0:
                    bcd = smp.tile([128, COUT], F32, name="bcd", tag="bcd")
                    nc.scalar.copy(bcd, pbc)
                    nc.sync.dma_start(out=dbg_bc[:, :], in_=bcd)
                # count[j] = #{p : sal_j < sal_p} via compare + ones-matmul
                pcnt = psp.tile([128, 512], F32, name=f"pcnt{b}", tag="bank")
                for c in range(NCOT):
                    cmp = smp.tile([128, COUT], BF16, name=f"cmp{b}{c}", tag="cmp", bufs=2)
                    nc.vector.tensor_scalar(cmp, pbc, sal_cb[c][:, b:b + 1],
                                            None, op0=ALU.is_lt)
                    nc.tensor.matmul(pcnt[0:1, :], ones1, cmp,
                                     start=(c == 0), stop=(c == NCOT - 1))
                # thr = min over {count<=409 ? sal : BIG}; counts read
                # straight from PSUM (exact f32 integers)
                t3 = smp.tile([1, COUT], F32, name=f"t3{b}", tag="t3", bufs=1)
                nc.vector.tensor_scalar(t3, pcnt[0:1, :], CR_KEEP, BIG,
                                        op0=ALU.is_ge, op1=ALU.mult)
                nc.vector.tensor_tensor(out=t3, in0=t3, in1=pbc[0:1, :],
                                        op=ALU.max)
                thr = smp.tile([1, 1], F32, name=f"thr{b}", tag="thr")
                nc.vector.tensor_reduce(thr, t3, axis=AX.X, op=ALU.min)
                # broadcast thr via a DRAM bounce (keeps the serial top-k
                # tail out of the PE queue), then build the mask directly in
                # channel-partition layout
                nc.scalar.dma_start(out=thr_d[b, :], in_=thr)
                nc.scalar.dma_start(
                    out=thrB[b],
                    in_=bass.AP(tensor=thr_d.tensor, offset=thr_d.offset + b,
                                ap=[[0, 128], [1, 1]]))
                for c in range(NCOT):
                    nc.vector.scalar_tensor_tensor(
                        out=mask_cb[:, c * NB + b: c * NB + b + 1],
                        in0=sal_cb[c][:, b:b + 1], scalar=thrB[b][:, 0:1],
                        in1=sal_cb[c][:, b:b + 1], op0=ALU.is_gt, op1=ALU.mult)


            # ---------- conv (cot-major) + per-cot BN allgather + epilogue ----
            cc_in = dp.tile([NCOT, 128, 2], F32, name="cc_in")
            s1 = [pp.tile([128, 2 * NB], F32, name=f"s1_{c}") for c in range(NCOT)]
            s2 = [pp.tile([128, 2 * NB], F32, name=f"s2_{c}") for c in range(NCOT)]
            inv_n = 1.0 / float(B * NSP)
            scl = pp.tile([128, NCOT], F32, name="scl")
            shf = pp.tile([128, NCOT], F32, name="shf")

            wchts = {0: wcht0, 1: wcht1}
            for cot in range(NCOT):
                yv = bigp.tile([128, NB * NSP], BF16, name=f"y{cot}", tag="big")
                if cot + 1 < NCOT:
                    wq.append(wqp.tile([128, 32 * 128], BF16, name=f"wq{cot + 1}",
                                       tag="wq"))
                for b in range(NB):
                    banks = [psp.tile([128, 512], F32, name=f"bk{cot}{b}{n}",
                                      tag="bank") for n in range(2)]
                    # x-gated units (cot 0, later samples) run bank-major so
                    # the lower output half proceeds before the sample's last
                    # x rows have even arrived (subtile deps track quad rows)
                    n_major = (cot == 0 and b >= 1)
                    total_mm = NT * 16 * 2
                    order = ([(n, kk) for n in range(2) for kk in KHW_ORDER]
                             if n_major else
                             [(n, kk) for kk in KHW_ORDER for n in range(2)])
                    n_mm = 0
                    bank_cnt = [0, 0]
                    for n, (kh, kw) in order:
                        ph, dj = PAR[kh]
                        pw, di = PAR[kw]
                        cl = max(0, -di)
                        ch_ = min(OW - 1, OW - 1 - di)
                        for t in range(NT):
                            g = (kh * KK + kw) * NT + t
                            lhsT = wq[cot][:, g * 128:(g + 1) * 128]
                            oh_lo = max(16 * n, -dj)
                            oh_hi = min(16 * n + 15, OH - 1 - dj)
                            rhs = quads[b][t][ph][pw][
                                :, oh_lo + dj: oh_hi + dj + 1,
                                cl + di: ch_ + di + 1]
                            dst = banks[n].rearrange(
                                "p (r c) -> p r c", r=16)[
                                :, oh_lo - 16 * n: oh_hi - 16 * n + 1,
                                cl: ch_ + 1]
                            nc.tensor.matmul(
                                dst, lhsT, rhs,
                                start=(bank_cnt[n] == 0),
                                stop=(bank_cnt[n] == NT * 16 - 1),
                                skip_group_check=True)
                            n_mm += 1
                            bank_cnt[n] += 1
                    # interleave: per-sample top-k (cot 0) and next-chunk
                    # ternarize+transposes, so the PE never waits on late data
                    if cot == 0:
                        topk_sample(b)
                        if b == 1:
                            wchts[2] = smp.tile([128, NT * 128 * 16], BF16,
                                                name="wcht2", tag="wcht", bufs=2)
                            tern_half(2, wchts[2], 0)
                            tern_half(2, wchts[2], 1)
                    if cot == 1 and b == 1:
                        wchts[3] = smp.tile([128, NT * 128 * 16], BF16,
                                            name="wcht3", tag="wcht", bufs=2)
                        tern_half(3, wchts[3], 0)
                        tern_half(3, wchts[3], 1)
                    if cot + 1 < NCOT:
                        transpose_gbs(cot + 1, wchts[cot + 1], wq[cot + 1],
                                      [2 * b, 2 * b + 1])
                    for n in range(2):
                        slot = b * 2 + n
                        nc.scalar.activation(
                            yv[:, b * NSP + n * 512: b * NSP + (n + 1) * 512],
                            banks[n], AF.Copy, bias=0.0,
                            scale=mask_cb[:, cot * NB + b: cot * NB + b + 1],
                            accum_out=s1[cot][:, slot:slot + 1])
                        sq = stp.tile([128, 512], BF16, name=f"sq{cot}{b}{n}",
                                      tag="sq", bufs=2)
                        nc.vector.scalar_tensor_tensor(
                            out=sq, in0=banks[n],
                            scalar=mask_cb[:, cot * NB + b: cot * NB + b + 1],
                            in1=yv[:, b * NSP + n * 512: b * NSP + (n + 1) * 512],
                            op0=ALU.mult, op1=ALU.mult,
                            accum_out=s2[cot][:, slot:slot + 1])

                # BN stats: local reduce -> AllGather -> global reduce
                r12 = smp.tile([128, 2], F32, name=f"r12_{cot}", tag="r12")
                nc.vector.tensor_reduce(r12[:, 0:1], s1[cot], axis=AX.X,
                                        op=ALU.add)
                nc.vector.tensor_reduce(r12[:, 1:2], s2[cot], axis=AX.X,
                                        op=ALU.add)
                nc.sync.dma_start(out=cc_in[cot, :, :], in_=r12)
                nc.gpsimd.collective_compute(
                    "AllGather", ALU.bypass,
                    replica_groups=[list(range(N_CORES))],
                    ins=[cc_in[cot, :, :]], outs=[cc_out[cot, :, :, :]])
                sg = smp.tile([128, 2 * N_CORES], F32, name=f"sg{cot}", tag="sg")
                nc.sync.dma_start(
                    out=sg, in_=bass.AP(tensor=cc_out, offset=cot * 2 * 128 * N_CORES,
                                        ap=[[2, 128], [256, N_CORES], [1, 2]]))
                s12 = smp.tile([128, 2], F32, name=f"s12_{cot}", tag="r12")
                sgv = bass.AP(tensor=sg.tensor, offset=sg.offset,
                              ap=[sg.ap[0], [1, 2], [2, N_CORES]])
                nc.vector.tensor_reduce(s12, sgv, axis=AX.X, op=ALU.add)
                mu = smp.tile([128, 1], F32, name=f"mu{cot}", tag="mu", bufs=1)
                nc.vector.tensor_scalar(mu, s12[:, 0:1], inv_n, None, op0=ALU.mult)
                m2 = smp.tile([128, 1], F32, name=f"m2{cot}", tag="m2", bufs=1)
                nc.vector.tensor_scalar(m2, s12[:, 1:2], inv_n, None, op0=ALU.mult)
                var = smp.tile([128, 1], F32, name=f"var{cot}", tag="var", bufs=1)
                nc.vector.scalar_tensor_tensor(
                    out=var, in0=mu, scalar=mu[:, :], in1=m2,
                    op0=ALU.mult, op1=ALU.subtract)  # mu*mu - m2 = -var
                sv = smp.tile([128, 1], F32, name=f"sv{cot}", tag="sv", bufs=1)
                nc.scalar.activation(sv, var, AF.Sqrt, bias=epst[:, :], scale=-1.0)
                rstd = smp.tile([128, 1], F32, name=f"rstd{cot}", tag="rstd", bufs=1)
                nc.vector.reciprocal(rstd, sv)
                nc.vector.tensor_tensor(out=scl[:, cot:cot + 1],
                                        in0=gam_t[:, cot:cot + 1], in1=rstd,
                                        op=ALU.mult)
                msc = smp.tile([128, 1], F32, name=f"msc{cot}", tag="msc", bufs=1)
                nc.vector.tensor_tensor(out=msc, in0=mu,
                                        in1=scl[:, cot:cot + 1], op=ALU.mult)
                nc.vector.tensor_tensor(out=shf[:, cot:cot + 1],
                                        in0=bet_t[:, cot:cot + 1], in1=msc,
                                        op=ALU.subtract)

                # epilogue: z = y*scl + shf ; out = max(z, 0.2*z).  Mid-conv
                # cots run it on the DVE (the Act engine must keep draining
                # the next cot's evictions); the last cot has no later
                # evictions, so split across Act (Prelu) + DVE for the tail.
                if cot == NCOT - 1:
                    # tail epilogue: Act and DVE in parallel, one fat DMA per
                    # sample (the x-staging ring is free by now)
                    for b in range(NB):
                        zt = stp.tile([128, NSP], F32, name=f"zt{b}",
                                      tag="stage")
                        for n in range(2):
                            zn = zt[:, n * 512:(n + 1) * 512]
                            ysl = yv[:, b * NSP + n * 512:
                                     b * NSP + (n + 1) * 512]
                            if n == 0:
                                nc.scalar.activation(
                                    zn, ysl, AF.Prelu,
                                    bias=shf[:, cot:cot + 1],
                                    scale=scl[:, cot:cot + 1],
                                    alpha=float(NEG_SLOPE))
                            else:
                                nc.vector.tensor_scalar(
                                    zn, ysl, scl[:, cot:cot + 1],
                                    shf[:, cot:cot + 1],
                                    op0=ALU.mult, op1=ALU.add)
                                nc.vector.scalar_tensor_tensor(
                                    out=zn, in0=zn, scalar=float(NEG_SLOPE),
                                    in1=zn, op0=ALU.mult, op1=ALU.max)
                        nc.sync.dma_start(
                            out=out[b, cot * 128:(cot + 1) * 128, :, :].rearrange(
                                "p h w -> p (h w)"),
                            in_=zt)
                else:
                    for b in range(NB):
                        for n in range(2):
                            z = stp.tile([128, 512], F32, name=f"z{cot}{b}{n}",
                                         tag="z", bufs=3)
                            ysl = yv[:, b * NSP + n * 512: b * NSP + (n + 1) * 512]
                            nc.vector.tensor_scalar(z, ysl, scl[:, cot:cot + 1],
                                                    shf[:, cot:cot + 1],
                                                    op0=ALU.mult, op1=ALU.add)
                            nc.vector.scalar_tensor_tensor(
                                out=z, in0=z, scalar=float(NEG_SLOPE), in1=z,
                                op0=ALU.mult, op1=ALU.max)
                            nc.sync.dma_start(
                                out=out[b, cot * 128:(cot + 1) * 128, :, :].rearrange(
                                    "p h w -> p (h w)")[:, n * 512:(n + 1) * 512],
                                in_=z)

            if debug:
                nc.sync.dma_start(out=dbg_mask[:, :], in_=mask_cb)
                for t in range(NT):
                    nc.sync.dma_start(out=bass.AP(tensor=dbg_sub, offset=t * NB,
                                                  ap=[[NT * NB, 128], [1, NB]]),
                                      in_=subT[t])
                for c in range(NCOT):
                    nc.sync.dma_start(out=bass.AP(tensor=dbg_sal, offset=c * NB,
                                                  ap=[[NCOT * NB, 128], [1, NB]]),
                                      in_=sal_cb[c])
                    nc.sync.dma_start(out=dbg_s1[c, :, :], in_=s1[c])
                    nc.sync.dma_start(out=dbg_s2[c, :, :], in_=s2[c])
                nc.sync.dma_start(out=dbg_thr[:],
                                  in_=bass.AP(tensor=thr_d.tensor,
                                              offset=thr_d.offset,
                                              ap=[[1, NB]]))
                nc.sync.dma_start(out=dbg_wq[:, :], in_=wq[3])
                nc.sync.dma_start(out=dbg_scl[:, :], in_=scl)
                nc.sync.dma_start(out=dbg_shf[:, :], in_=shf)

    _split_waits(nc)
    return nc


_CACHE = {}


def kernel(x, weight, pos, neg, sal_w, sal_b, gamma, beta):
    x = np.ascontiguousarray(np.asarray(x, dtype=np.float32))
    weight = np.ascontiguousarray(np.asarray(weight, dtype=np.float32))
    sal_w = np.ascontiguousarray(np.asarray(sal_w, dtype=np.float32))
    sal_b = np.ascontiguousarray(np.asarray(sal_b, dtype=np.float32))
    gamma = np.ascontiguousarray(np.asarray(gamma, dtype=np.float32))
    beta = np.ascontiguousarray(np.asarray(beta, dtype=np.float32))
    pos_f = np.float32(np.asarray(pos).reshape(()))
    neg_f = np.float32(np.asarray(neg).reshape(()))

    r_imm = float(np.float32(neg_f / pos_f))
    eps_imm = float(np.float32(BN_EPS) / (pos_f * pos_f))

    import os
    debug = os.environ.get("KERNEL_DEBUG", "0") == "1"
    key = (r_imm, eps_imm, debug)
    if key not in _CACHE:
        _CACHE[key] = build_kernel(r_imm, eps_imm, debug)
    nc = _CACHE[key]

    in_maps = []
    for c in range(N_CORES):
        in_maps.append({
            "xs": x[c * NB:(c + 1) * NB],
            "wt": weight,
            "salw": sal_w,
            "salb": sal_b,
            "gam": gamma,
            "bet": beta,
        })
    res = run_bass_kernel_spmd(nc, in_maps, core_ids=list(range(N_CORES)))
    if debug:
        kernel.dbg = res.results
    out = np.concatenate([res.results[c]["out"] for c in range(N_CORES)], axis=0)
    return out
